# revision 19
# baseline (speedup 1.0000x reference)
"""Trainium2 Bass kernel for ApertureChamberSSM (v3).

Computation (reference):
    iv, ov, beta_s, alpha, mg = sigmoid(scalars); decay = exp(-alpha)
    x_in  = iv * x ; drive = tanh(x_in)
    psi_s = decay * psi_{s-1} + (1-decay) * drive_s          (scan over S)
    x_mem = mg * psi + (1-mg) * x_in
    rotate channel pairs (j, j+512) by pi*sigmoid(beta), scale by ov

Algebra: psi = (1-decay)*psi' with psi'_s = decay*psi'_{s-1} + drive_s
    x_mem = ap_*psi' + c*x   (ap_ = mg*(1-decay), c = (1-mg)*iv)
    out   = R @ x_mem        (per channel pair, R = ov*[[cos,-sin],[sin,cos]])
          = (ap_*R) @ psi' + (c*R) @ x

Layout: each SBUF tile holds 64 real channels (partitions 0..63) and their
64 paired imag channels (partitions 64..127), so the rotation+blend is TWO
dense 128x128 bf16 matmuls per PSUM tile (lhsT = (ap_*R)^T and (c*R)^T),
accumulated on the TensorEngine.

Per core (8 cores, zero communication): 4 row-tiles (one per batch) x 8
seq-chunks of 1024. DMA bf16 in/out; tanh on ACT (f32 drive);
tensor_tensor_scan on DVE (bf16 psi); 2 matmuls per 512-sub on PE;
PSUM->SBUF bf16 eviction on ACT. Host does sigmoid/cos/sin, sharding
transposes, and f32 upcast.

Sharding: core c owns channel pairs j in [64c, 64c+64) for all 4 batches.
Shard (512, 8192) rows: tile b in [0,4): rows [128b,128b+64) = real
channels j (seq-major), rows [128b+64,128b+128) = imag channels j+512.
"""

import math

import numpy as np

B, S, D = 4, 8192, 1024
HALF = D // 2          # 512
NCORES = 8
JPC = HALF // NCORES   # 64 channel pairs per core
ROWS = 2 * B * JPC     # 512 rows per core
P = 128                # partitions
C = 2048               # seq chunk (free dim) per tile
NCHUNK = S // C
NTILE = ROWS // P      # 4 row-tiles per core (one per batch)
MMF = 512              # matmul moving free dim (one PSUM bank)

_cache = {}


def _sig(v):
    return 1.0 / (1.0 + math.exp(-float(v)))


def _build(iv, decay, use_scan):
    """Build + compile the 8-core SPMD graph. The rotation/blend matrices
    arrive at runtime via the 'consts' input, so only iv, decay and
    use_scan are baked in."""
    import concourse.bass as bass
    import concourse.tile as tile
    from concourse import bacc, mybir

    f32 = mybir.dt.float32
    bf16 = mybir.dt.bfloat16
    AF = mybir.ActivationFunctionType
    OP = mybir.AluOpType

    nc = bacc.Bacc("TRN2", target_bir_lowering=False, debug=False,
                   num_devices=NCORES)
    x_ap = nc.dram_tensor("x", [ROWS, S], bf16, kind="ExternalInput").ap()
    consts_ap = nc.dram_tensor("consts", [P, 2 * P], bf16,
                               kind="ExternalInput").ap()
    out_ap = nc.dram_tensor("out", [ROWS, S], bf16, kind="ExternalOutput").ap()

    with tile.TileContext(nc) as tc:
        with (
            tc.tile_pool(name="const", bufs=1) as cpool,
            tc.tile_pool(name="xin", bufs=3) as xpool,
            tc.tile_pool(name="drv", bufs=3) as dpool,
            tc.tile_pool(name="psi", bufs=3) as ppool,
            tc.tile_pool(name="outs", bufs=3) as opool,
            tc.tile_pool(name="ps", bufs=1, space=bass.MemorySpace.PSUM) as pspool,
        ):
            idm = cpool.tile([P, 2 * P], bf16, tag="idm")
            nc.sync.dma_start(idm[:], consts_ap[:])
            lhs_psi = idm[:, 0:P]      # (ap_*R)^T
            lhs_x = idm[:, P:2 * P]    # (c*R)^T

            if use_scan:
                # preload the Tanh LUT while input DMAs stream
                warm = cpool.tile([P, 1], bf16, tag="warm")
                nc.gpsimd.memset(warm[:], 0.0)
                nc.scalar.activation(warm[:], warm[:], AF.Tanh,
                                     bias=0.0, scale=iv)
                dk = cpool.tile([P, C], f32, tag="dk")
                nc.gpsimd.memset(dk[:], decay)

            prev = [None] * NTILE

            def front(off, w, b):
                r0 = b * P
                x_t = xpool.tile([P, C], bf16, tag=f"x{b}")
                nc.sync.dma_start(
                    x_t[:, 0:w], x_ap[r0:r0 + P, off:off + w])
                p_t = None
                if use_scan:
                    d_t = dpool.tile([P, C], bf16, tag=f"d{b}")
                    nc.scalar.activation(d_t[:, 0:w], x_t[:, 0:w], AF.Tanh,
                                         bias=0.0, scale=iv)
                    p_t = ppool.tile([P, C], bf16, tag=f"p{b}")
                    init = prev[b] if prev[b] is not None else 0.0
                    nc.vector.tensor_tensor_scan(
                        p_t[:, 0:w], dk[:, 0:w], d_t[:, 0:w], init,
                        OP.mult, OP.add)
                    prev[b] = p_t[:, w - 1:w]
                return x_t, p_t

            def back(off, w, b, x_t, p_t):
                r0 = b * P
                o_t = opool.tile([P, C], bf16, tag=f"o{b}")
                ps = pspool.tile([P, C], f32, tag=f"ps{b % 2}")
                pairs = ([(lhs_psi, p_t), (lhs_x, x_t)] if use_scan
                         else [(lhs_x, x_t)])
                for t, (lw, src) in enumerate(pairs):
                    for s4 in range(w // MMF):
                        fs = slice(s4 * MMF, (s4 + 1) * MMF)
                        nc.tensor.matmul(
                            ps[:, fs], lw, src[:, fs],
                            start=(t == 0), stop=(t == len(pairs) - 1))
                nc.scalar.copy(o_t[:, 0:w], ps[:, 0:w])
                nc.sync.dma_start(
                    out_ap[r0:r0 + P, off:off + w], o_t[:, 0:w])

            widths = [512, 1536, 2048, 2048, 1536, 512]
            assert sum(widths) == S
            pend = None
            off = 0
            for w in widths:
                for b in range(NTILE):
                    cur = front(off, w, b)
                    if pend is not None:
                        back(*pend)
                    pend = (off, w, b, *cur)
                off += w
            back(*pend)

    nc.compile()
    return nc


def kernel(x, beta, input_valve, output_valve, alpha_raw, memory_gate):
    x = np.asarray(x, dtype=np.float32)
    assert x.shape == (B, S, D), x.shape

    beta_s = _sig(beta)
    iv = _sig(input_valve)
    ov = _sig(output_valve)
    alpha = _sig(alpha_raw)
    mg = _sig(memory_gate)
    decay = math.exp(-alpha)
    c = (1.0 - mg) * iv
    ap_ = mg * (1.0 - decay)
    angle = math.pi * beta_s
    p_, q_ = math.cos(angle) * ov, math.sin(angle) * ov
    use_scan = ap_ != 0.0

    key = (round(iv, 12), round(decay, 12), use_scan)
    if key not in _cache:
        _cache[key] = _build(iv, decay, use_scan)
    nc = _cache[key]

    import ml_dtypes
    from concourse.bass_utils import run_bass_kernel_spmd

    bf = ml_dtypes.bfloat16
    # R = ov * [[cos, -sin],[sin, cos]] acting on (real_j, imag_j) pairs at
    # partitions (m, 64+m).  lhsT[k, m] = R[m, k].
    h = P // 2
    eye = np.eye(h, dtype=np.float64)
    Rt = np.zeros((P, P))
    Rt[:h, :h] = p_ * eye
    Rt[:h, h:] = q_ * eye
    Rt[h:, :h] = -q_ * eye
    Rt[h:, h:] = p_ * eye
    consts = np.concatenate([ap_ * Rt, c * Rt], axis=1).astype(bf)

    xr = x[:, :, :HALF].reshape(B, S, NCORES, JPC)
    xi = x[:, :, HALF:].reshape(B, S, NCORES, JPC)
    in_maps = []
    for cix in range(NCORES):
        shard = np.empty((NTILE, 2, JPC, S), dtype=bf)
        for b in range(B):
            shard[b, 0] = xr[b, :, cix, :].T.astype(bf)
            shard[b, 1] = xi[b, :, cix, :].T.astype(bf)
        in_maps.append({"x": shard.reshape(ROWS, S), "consts": consts})

    res = run_bass_kernel_spmd(nc, in_maps, core_ids=list(range(NCORES)))
    global last_result
    last_result = res

    out = np.empty((B, S, D), dtype=np.float32)
    o_r = out[:, :, :HALF].reshape(B, S, NCORES, JPC)
    o_i = out[:, :, HALF:].reshape(B, S, NCORES, JPC)
    for cix in range(NCORES):
        oc = np.asarray(res.results[cix]["out"]).reshape(NTILE, 2, JPC, S)
        for b in range(B):
            o_r[b, :, cix, :] = oc[b, 0].T.astype(np.float32)
            o_i[b, :, cix, :] = oc[b, 1].T.astype(np.float32)
    return out


# revision 20
# speedup vs baseline: 1.0233x; 1.0233x over previous
"""Trainium2 Bass kernel for ApertureChamberSSM (v3).

Computation (reference):
    iv, ov, beta_s, alpha, mg = sigmoid(scalars); decay = exp(-alpha)
    x_in  = iv * x ; drive = tanh(x_in)
    psi_s = decay * psi_{s-1} + (1-decay) * drive_s          (scan over S)
    x_mem = mg * psi + (1-mg) * x_in
    rotate channel pairs (j, j+512) by pi*sigmoid(beta), scale by ov

Algebra: psi = (1-decay)*psi' with psi'_s = decay*psi'_{s-1} + drive_s
    x_mem = ap_*psi' + c*x   (ap_ = mg*(1-decay), c = (1-mg)*iv)
    out   = R @ x_mem        (per channel pair, R = ov*[[cos,-sin],[sin,cos]])
          = (ap_*R) @ psi' + (c*R) @ x

Layout: each SBUF tile holds 64 real channels (partitions 0..63) and their
64 paired imag channels (partitions 64..127), so the rotation+blend is TWO
dense 128x128 bf16 matmuls per PSUM tile (lhsT = (ap_*R)^T and (c*R)^T),
accumulated on the TensorEngine.

Per core (8 cores, zero communication): 4 row-tiles (one per batch) x 8
seq-chunks of 1024. DMA bf16 in/out; tanh on ACT (f32 drive);
tensor_tensor_scan on DVE (bf16 psi); 2 matmuls per 512-sub on PE;
PSUM->SBUF bf16 eviction on ACT. Host does sigmoid/cos/sin, sharding
transposes, and f32 upcast.

Sharding: core c owns channel pairs j in [64c, 64c+64) for all 4 batches.
Shard (512, 8192) rows: tile b in [0,4): rows [128b,128b+64) = real
channels j (seq-major), rows [128b+64,128b+128) = imag channels j+512.
"""

import math

import numpy as np

B, S, D = 4, 8192, 1024
HALF = D // 2          # 512
NCORES = 8
JPC = HALF // NCORES   # 64 channel pairs per core
ROWS = 2 * B * JPC     # 512 rows per core
P = 128                # partitions
C = 2048               # seq chunk (free dim) per tile
NCHUNK = S // C
NTILE = ROWS // P      # 4 row-tiles per core (one per batch)
MMF = 512              # matmul moving free dim (one PSUM bank)

_cache = {}


def _sig(v):
    return 1.0 / (1.0 + math.exp(-float(v)))


def _build(iv, decay, use_scan):
    """Build + compile the 8-core SPMD graph. The rotation/blend matrices
    arrive at runtime via the 'consts' input, so only iv, decay and
    use_scan are baked in."""
    import concourse.bass as bass
    import concourse.tile as tile
    from concourse import bacc, mybir

    f32 = mybir.dt.float32
    bf16 = mybir.dt.bfloat16
    AF = mybir.ActivationFunctionType
    OP = mybir.AluOpType

    nc = bacc.Bacc("TRN2", target_bir_lowering=False, debug=False,
                   num_devices=NCORES)
    x_ap = nc.dram_tensor("x", [ROWS, S], bf16, kind="ExternalInput").ap()
    consts_ap = nc.dram_tensor("consts", [P, 2 * P], bf16,
                               kind="ExternalInput").ap()
    out_ap = nc.dram_tensor("out", [ROWS, S], bf16, kind="ExternalOutput").ap()

    with tile.TileContext(nc) as tc:
        with (
            tc.tile_pool(name="const", bufs=1) as cpool,
            tc.tile_pool(name="xin", bufs=3) as xpool,
            tc.tile_pool(name="drv", bufs=3) as dpool,
            tc.tile_pool(name="psi", bufs=3) as ppool,
            tc.tile_pool(name="outs", bufs=3) as opool,
            tc.tile_pool(name="ps", bufs=1, space=bass.MemorySpace.PSUM) as pspool,
        ):
            idm = cpool.tile([P, 2 * P], bf16, tag="idm")
            nc.sync.dma_start(idm[:], consts_ap[:])
            lhs_psi = idm[:, 0:P]      # (ap_*R)^T
            lhs_x = idm[:, P:2 * P]    # (c*R)^T

            if use_scan:
                # preload the Tanh LUT while input DMAs stream
                warm = cpool.tile([P, 1], bf16, tag="warm")
                nc.gpsimd.memset(warm[:], 0.0)
                nc.scalar.activation(warm[:], warm[:], AF.Tanh,
                                     bias=0.0, scale=iv)
                dk = cpool.tile([P, C], f32, tag="dk")
                nc.gpsimd.memset(dk[:], decay)

            prev = [None] * NTILE

            def front(off, w, b):
                r0 = b * P
                x_t = xpool.tile([P, C], bf16, tag=f"x{b}")
                nc.sync.dma_start(
                    x_t[:, 0:w], x_ap[r0:r0 + P, off:off + w])
                p_t = None
                if use_scan:
                    d_t = dpool.tile([P, C], bf16, tag=f"d{b}")
                    nc.scalar.activation(d_t[:, 0:w], x_t[:, 0:w], AF.Tanh,
                                         bias=0.0, scale=iv)
                    p_t = ppool.tile([P, C], bf16, tag=f"p{b}")
                    init = prev[b] if prev[b] is not None else 0.0
                    nc.vector.tensor_tensor_scan(
                        p_t[:, 0:w], dk[:, 0:w], d_t[:, 0:w], init,
                        OP.mult, OP.add)
                    prev[b] = p_t[:, w - 1:w]
                return x_t, p_t

            def back(off, w, b, x_t, p_t):
                r0 = b * P
                o_t = opool.tile([P, C], bf16, tag=f"o{b}")
                ps = pspool.tile([P, C], f32, tag=f"ps{b % 2}")
                pairs = ([(lhs_psi, p_t), (lhs_x, x_t)] if use_scan
                         else [(lhs_x, x_t)])
                for t, (lw, src) in enumerate(pairs):
                    for s4 in range(w // MMF):
                        fs = slice(s4 * MMF, (s4 + 1) * MMF)
                        nc.tensor.matmul(
                            ps[:, fs], lw, src[:, fs],
                            start=(t == 0), stop=(t == len(pairs) - 1))
                nc.scalar.copy(o_t[:, 0:w], ps[:, 0:w])
                nc.sync.dma_start(
                    out_ap[r0:r0 + P, off:off + w], o_t[:, 0:w])

            widths = [C] * NCHUNK
            assert sum(widths) == S
            pend = None
            off = 0
            for w in widths:
                for b in range(NTILE):
                    cur = front(off, w, b)
                    if pend is not None:
                        back(*pend)
                    pend = (off, w, b, *cur)
                off += w
            back(*pend)

    nc.compile()
    return nc


def kernel(x, beta, input_valve, output_valve, alpha_raw, memory_gate):
    x = np.asarray(x, dtype=np.float32)
    assert x.shape == (B, S, D), x.shape

    beta_s = _sig(beta)
    iv = _sig(input_valve)
    ov = _sig(output_valve)
    alpha = _sig(alpha_raw)
    mg = _sig(memory_gate)
    decay = math.exp(-alpha)
    c = (1.0 - mg) * iv
    ap_ = mg * (1.0 - decay)
    angle = math.pi * beta_s
    p_, q_ = math.cos(angle) * ov, math.sin(angle) * ov
    use_scan = ap_ != 0.0

    key = (round(iv, 12), round(decay, 12), use_scan)
    if key not in _cache:
        _cache[key] = _build(iv, decay, use_scan)
    nc = _cache[key]

    import ml_dtypes
    from concourse.bass_utils import run_bass_kernel_spmd

    bf = ml_dtypes.bfloat16
    # R = ov * [[cos, -sin],[sin, cos]] acting on (real_j, imag_j) pairs at
    # partitions (m, 64+m).  lhsT[k, m] = R[m, k].
    h = P // 2
    eye = np.eye(h, dtype=np.float64)
    Rt = np.zeros((P, P))
    Rt[:h, :h] = p_ * eye
    Rt[:h, h:] = q_ * eye
    Rt[h:, :h] = -q_ * eye
    Rt[h:, h:] = p_ * eye
    consts = np.concatenate([ap_ * Rt, c * Rt], axis=1).astype(bf)

    xr = x[:, :, :HALF].reshape(B, S, NCORES, JPC)
    xi = x[:, :, HALF:].reshape(B, S, NCORES, JPC)
    in_maps = []
    for cix in range(NCORES):
        shard = np.empty((NTILE, 2, JPC, S), dtype=bf)
        for b in range(B):
            shard[b, 0] = xr[b, :, cix, :].T.astype(bf)
            shard[b, 1] = xi[b, :, cix, :].T.astype(bf)
        in_maps.append({"x": shard.reshape(ROWS, S), "consts": consts})

    res = run_bass_kernel_spmd(nc, in_maps, core_ids=list(range(NCORES)))
    global last_result
    last_result = res

    out = np.empty((B, S, D), dtype=np.float32)
    o_r = out[:, :, :HALF].reshape(B, S, NCORES, JPC)
    o_i = out[:, :, HALF:].reshape(B, S, NCORES, JPC)
    for cix in range(NCORES):
        oc = np.asarray(res.results[cix]["out"]).reshape(NTILE, 2, JPC, S)
        for b in range(B):
            o_r[b, :, cix, :] = oc[b, 0].T.astype(np.float32)
            o_i[b, :, cix, :] = oc[b, 1].T.astype(np.float32)
    return out
